# revision 37
# baseline (speedup 1.0000x reference)
# Linformer attention kernel for Trainium2 (8 NeuronCores, SPMD, no collectives).
#
# Sharding: core c = (batch b = c//2, head-group hg = c%2). Each core runs one
# batch's Linformer attention for 8 of the 16 heads: Wq/Wk/Wv column-sliced
# (512 cols), Wo row-sliced -> partial [D, L] output (transposed); host sums
# the two partials per batch and transposes back.
#
# Key algebraic restructure: the Linformer low-rank projection commutes with
# the K/V linears: k_proj = Wk^T (x^T pk). So we never materialize full K/V:
# compute xkv^T = x^T [pk || pv] then two small [K, D] GEMMs instead of two
# full [L, DG] GEMMs + reduces.
#
# v5 vs the original baseline: xn/pkv tile DMAs lead each lb and the xkv
# matmuls are emitted before the qT matmuls, so the PE starts ~2us into the
# kernel instead of ~30us; wq loads right after lb0's tiles and wk/wv/wo
# stream in small chunks across lbs 1..6 (they are needed only at the
# epilogue); the qT path (xT, wq) runs in bf16, halving the biggest input
# stream; the yT output is bf16, halving the output stream (host sums the
# two partials in f32).

import os
import sys

for _p in ("/opt/trn_rl_repo", "/root/.axon_site/_ro/trn_rl_repo"):
    if os.path.isdir(_p) and _p not in sys.path:
        sys.path.insert(0, _p)
        break

import numpy as np

import concourse.bass as bass
from concourse import bacc
import concourse.mybir as mybir
from concourse.bass_utils import run_bass_kernel_spmd
from concourse.tile import TileContext

P = 128
B, L, D = 4, 4096, 1024
H, HD = 16, 64
K = 256
SCALE = HD ** -0.5

NCORES = 8
HG = 2                 # head groups (cores per batch)
DG = D // HG           # 512: local width of Wq/Wk/Wv cols & Wo rows
HL = H // HG           # 8 local heads
KC = D // P            # 8 contraction chunks over D
DGT = DG // P          # 4 partition-tiles over local d
KT = K // P            # 2 partition-tiles over low-rank K
K2 = 2 * K             # 512: pk || pv concat width
LB = 512               # L block
NLB = L // LB          # 8

f32 = mybir.dt.float32
f32r = mybir.dt.float32r
bf16 = mybir.dt.bfloat16
AF = mybir.ActivationFunctionType


def build_kernel(nc: bass.Bass):
    xT = nc.dram_tensor("xT", (D, L), bf16, kind="ExternalInput")
    xn = nc.dram_tensor("xn", (L, D), bf16, kind="ExternalInput")
    wq = nc.dram_tensor("wq", (D, DG), bf16, kind="ExternalInput")
    wk = nc.dram_tensor("wk", (D, DG), bf16, kind="ExternalInput")
    wv = nc.dram_tensor("wv", (D, DG), bf16, kind="ExternalInput")
    wo = nc.dram_tensor("wo", (DG, D), bf16, kind="ExternalInput")
    pkv = nc.dram_tensor("pkv", (L, K2), bf16, kind="ExternalInput")
    yT = nc.dram_tensor("yT", (D, L), bf16, kind="ExternalOutput")

    xn_r = xn.rearrange("(lt p) c -> lt p c", p=P)      # [32,128,1024]
    pkv_r = pkv.rearrange("(lt p) k -> lt p k", p=P)    # [32,128,512]
    yT_r = yT.rearrange("(nt p) l -> nt p l", p=P)      # [8,128,4096]
    xT_r = xT.rearrange("(kc p) l -> p kc l", p=P)      # [128,8,4096]
    wk_r = wk.rearrange("(kc p) m -> p kc m", p=P)
    wv_r = wv.rearrange("(kc p) m -> p kc m", p=P)
    wo_r = wo.rearrange("(dt p) n -> p dt n", p=P)

    with TileContext(nc) as tc:
        with tc.tile_pool(name="const", bufs=1) as cpool:
            # Resident tensors (per-partition bytes in comments).
            wq_sb = cpool.tile([P, KC, DG], bf16)        # 8K
            wk_sb = cpool.tile([P, KC, DG], bf16)        # 8K
            wv_sb = cpool.tile([P, KC, DG], bf16)        # 8K
            wo_sb = cpool.tile([P, DGT, D], bf16)        # 8K
            qT_sb = cpool.tile([P, DGT, L], bf16)        # 32K
            xkvT_sb = cpool.tile([P, KC, K2], f32r)      # 16K  x^T [pk||pv]
            xkv_bf = cpool.tile([P, KC, K2], bf16)       # 8K   bf16 copy
            kpT_pad = cpool.tile([P, HL, K], bf16)       # 4K   zero-padded/head
            vpa_sb = cpool.tile([P, KT, HL, P], bf16)    # 4K per-head slabs
            ones_c = cpool.tile([P, 1], bf16)            # denom stationary col
            scr = cpool.tile([P, 16], f32)               # gpsimd lib warmup
            nc.vector.memset(kpT_pad[:].bitcast(bf16), 0.0)
            nc.vector.memset(vpa_sb[:].bitcast(bf16), 0.0)
            nc.vector.memset(ones_c[:].bitcast(bf16), 1.0)
            nc.vector.memset(scr[:].bitcast(f32), 1.0)
            # dummy broadcast: forces the gpsimd ucode library load at t=0
            # (otherwise LOAD_LIB stalls the first real broadcast ~7us at
            # the phase-1 -> phase-2 transition)
            nc.gpsimd.partition_broadcast(scr[:, :], scr[0:1, :])
            # odd heads (values@p64..127): ones col 0 -> denom@p0 for free.
            # Even heads (values@p0..63) get a separate denominator matmul on
            # the PE: its partition-0 output feeds the reciprocal directly
            # (PE has slack in phase 2; DVE is the binding engine there, and
            # the ucode recip/broadcast only accept partition-0 operands).
            for kpt in range(KT):
                for h in range(1, HL, 2):
                    nc.vector.memset(
                        vpa_sb[:, kpt, h, 0:1].bitcast(bf16), 1.0)

            # ---------------- Phase 1: xkvT accumulation + qT ---------------
            with tc.tile_pool(name="p1_mm", bufs=2, space="PSUM") as mmp, \
                 tc.tile_pool(name="p1_xk", bufs=2, space="PSUM") as xkp, \
                 tc.tile_pool(name="p1_x", bufs=2) as xsp, \
                 tc.tile_pool(name="p1_xn", bufs=18) as xnp, \
                 tc.tile_pool(name="p1_p", bufs=18) as psp:
                xns = []
                pkvs = []
                for lb in range(NLB):
                    sl = slice(lb * LB, (lb + 1) * LB)

                    # DMA order: xn/pkv first (xkv matmuls start ~2us in),
                    # then wq + the xT block for the qT matmuls.
                    for j in range(LB // P):
                        lt = lb * (LB // P) + j
                        xnt = xnp.tile([P, D], bf16, tag="xn")
                        nc.sync.dma_start(xnt, xn_r[lt])
                        xns.append(xnt)
                        pkvt = psp.tile([P, K2], bf16, tag="pkv")
                        nc.sync.dma_start(pkvt, pkv_r[lt])
                        pkvs.append(pkvt)
                    if lb == 0:
                        nc.sync.dma_start(
                            wq_sb[:], wq.rearrange("(kc p) m -> p kc m", p=P))
                    xTb = xsp.tile([P, KC, LB], bf16, tag="xTb")
                    for kc2 in range(0, KC, 4):
                        nc.sync.dma_start(
                            xTb[:, kc2:kc2 + 4], xT_r[:, kc2:kc2 + 4, sl])
                    # wk/wv/wo are needed only at the epilogue: stream them
                    # in small chunks across lbs 1..6 so they never
                    # head-block the x/pkv tile stream.
                    if 1 <= lb <= 6:
                        wchunks = [(wk_sb, wk_r, 0, 4), (wk_sb, wk_r, 4, 8),
                                   (wv_sb, wv_r, 0, 4), (wv_sb, wv_r, 4, 8),
                                   (wo_sb, wo_r, 0, 2), (wo_sb, wo_r, 2, 4)]
                        dst, src, c0, c1 = wchunks[lb - 1]
                        for c in range(c0, c1):
                            nc.sync.dma_start(dst[:, c:c + 1], src[:, c:c + 1])

                    # xkvT[c, :] += x^T @ [pk || pv]: the PSUM chain spans an
                    # lb PAIR (8 matmuls) on odd lbs, halving both the chain
                    # boundaries and the DVE evacuation adds.
                    if lb % 2 == 1:
                        for cc in range(KC):
                            xk_ps = xkp.tile([P, K2], f32, tag="xk")
                            for j in range(2 * (LB // P)):
                                nc.tensor.matmul(
                                    xk_ps,
                                    (xns[j][:, cc * P:(cc + 1) * P]),
                                    (pkvs[j]),
                                    start=(j == 0),
                                    stop=(j == 2 * (LB // P) - 1),
                                )
                            if lb == 1:
                                nc.vector.tensor_copy(xkvT_sb[:, cc], xk_ps)
                            elif lb < NLB - 1:
                                nc.vector.tensor_add(
                                    xkvT_sb[:, cc],
                                    xkvT_sb[:, cc].bitcast(f32), xk_ps)
                            else:
                                # last add writes the bf16 copy directly
                                # (kp/vp matmuls consume bf16 operands)
                                nc.vector.tensor_add(
                                    xkv_bf[:, cc],
                                    xkvT_sb[:, cc].bitcast(f32), xk_ps)
                        xns = []
                        pkvs = []

                    # kp/vp epilogue runs BEFORE the last lb's qT matmuls so
                    # its DVE copies and the phase-2 pool handoff hide under
                    # the qT chains instead of stalling the PE.
                    if lb == NLB - 1:
                        # kpT[dt] = Wk-chunk^T @ xkT -> zero-padded per head
                        for dt_ in range(DGT):
                            kp_ps = mmp.tile([P, K], f32, tag="kp")
                            for cc in range(KC):
                                nc.tensor.matmul(
                                    kp_ps,
                                    (wk_sb[:, cc, dt_ * P:(dt_ + 1) * P]),
                                    (xkv_bf[:, cc, 0:K]),
                                    start=(cc == 0), stop=(cc == KC - 1),
                                )
                            for hh in range(2):
                                h = dt_ * 2 + hh
                                off = hh * 64
                                nc.vector.tensor_copy(
                                    kpT_pad[off:off + 64, h, :],
                                    kp_ps[off:off + 64])

                        # v_proj[k-half] = xvT^T @ Wv -> per-head padded
                        for kh in range(KT):
                            vp_ps = mmp.tile([P, DG], f32, tag="vp")
                            for cc in range(KC):
                                nc.tensor.matmul(
                                    vp_ps,
                                    (xkv_bf[:, cc, K + kh * P:K + (kh + 1) * P]),
                                    (wv_sb[:, cc]),
                                    start=(cc == 0), stop=(cc == KC - 1),
                                )
                            for h in range(HL):
                                off = (h % 2) * 64
                                nc.vector.tensor_copy(
                                    vpa_sb[:, kh, h, off:off + HD],
                                    vp_ps[:, h * HD:(h + 1) * HD],
                                )

                    # qT[dpt, lb-block] = (Wq columns)^T @ x^T-block
                    for dt_ in range(DGT):
                        q_ps = mmp.tile([P, LB], f32, tag="mm")
                        for kc in range(KC):
                            nc.tensor.matmul(
                                q_ps,
                                (wq_sb[:, kc, dt_ * P:(dt_ + 1) * P]),
                                (xTb[:, kc]),
                                start=(kc == 0), stop=(kc == KC - 1),
                            )
                        nc.scalar.copy(qT_sb[:, dt_, sl], q_ps)

            # ---------------- Phase 2+3: attention + output projection ------
            with tc.tile_pool(name="p2_lg", bufs=2, space="PSUM") as lgp, \
                 tc.tile_pool(name="p2_av", bufs=3, space="PSUM") as avp, \
                 tc.tile_pool(name="p2_dn", bufs=1, space="PSUM") as dnp, \
                 tc.tile_pool(name="p2_y", bufs=2, space="PSUM") as uyp, \
                 tc.tile_pool(name="p2_e", bufs=4) as ep2, \
                 tc.tile_pool(name="p2_o", bufs=2) as op2, \
                 tc.tile_pool(name="p2_r", bufs=4) as rp2, \
                 tc.tile_pool(name="p2_ys", bufs=4) as ysp:
                out_prev = None
                sl_prev = None
                for lb in range(NLB):
                    sl = slice(lb * LB, (lb + 1) * LB)
                    out_blk = op2.tile([P, DGT, LB], bf16, tag="oblk")

                    # software-pipelined: logits(h) issued on PE before
                    # attn@v(h-1); Wo chains of the PREVIOUS lb are threaded
                    # through this head loop (one n-chunk per head) so the
                    # normalization tail of head h never stalls the PE.
                    e_tiles = {}
                    for h in range(HL + 1):
                        if h < HL:
                            dt_ = h // 2
                            es = []
                            for kpt in range(KT):
                                lg_ps = lgp.tile([P, LB], f32, tag="lg")
                                nc.tensor.matmul(
                                    lg_ps,
                                    (kpT_pad[:, h, kpt * P:(kpt + 1) * P]),
                                    (qT_sb[:, dt_, sl]),
                                    start=True, stop=True,
                                )
                                e_sb = ep2.tile([P, LB], bf16, tag="eT")
                                nc.scalar.activation(e_sb, lg_ps, AF.Exp)
                                es.append(e_sb)
                            e_tiles[h] = es
                        if h > 0:
                            hp = h - 1
                            dtp = hp // 2
                            off = (hp % 2) * 64
                            es = e_tiles.pop(hp)
                            av_ps = avp.tile([P, LB], f32, tag="av")
                            for kpt in range(KT):
                                nc.tensor.matmul(
                                    av_ps,
                                    (vpa_sb[:, kpt, hp]),
                                    (es[kpt]),
                                    start=(kpt == 0), stop=(kpt == KT - 1),
                                )
                            if off == 0:
                                # even head: values@p0..63; denom via a
                                # separate PE matmul (p0 output, ucode-safe)
                                dn_ps = dnp.tile([1, LB], f32, tag="dn")
                                for kpt in range(KT):
                                    nc.tensor.matmul(
                                        dn_ps,
                                        (ones_c),
                                        (es[kpt]),
                                        start=(kpt == 0), stop=(kpt == KT - 1),
                                    )
                                den_ap = dn_ps[0:1]
                            else:
                                # odd head: ones col in vpa -> denom@p0
                                den_ap = av_ps[0:1]
                            rT2 = rp2.tile([P, LB], f32, tag="r2")
                            nc.vector.reciprocal_approx_fast(
                                rT2[0:1, :], den_ap)
                            rb = rp2.tile([P, LB], f32, tag="rb")
                            nc.gpsimd.partition_broadcast(rb[:, :], rT2[0:1, :])
                            nc.vector.tensor_tensor(
                                out_blk[off:off + 64, dtp],
                                av_ps[off:off + 64],
                                rb[off:off + 64],
                                mybir.AluOpType.mult,
                            )
                        # interleave one Wo chain of the prev lb; shifted one
                        # slot late (n = h-1) so the first chain doesn't wait
                        # on the tail of out_prev's normalization
                        if out_prev is not None:
                            n = h - 1 if h >= 1 else None
                            if n is not None:
                                y_ps = uyp.tile([P, LB], f32, tag="yps")
                                for dt_ in range(DGT):
                                    nc.tensor.matmul(
                                        y_ps,
                                        (wo_sb[:, dt_, n * P:(n + 1) * P]),
                                        (out_prev[:, dt_]),
                                        start=(dt_ == 0), stop=(dt_ == DGT - 1),
                                    )
                                y_sb = ysp.tile([P, LB], bf16, tag="ysb")
                                if n % 2 == 0:
                                    nc.scalar.copy(y_sb, y_ps)
                                else:
                                    nc.vector.tensor_copy(y_sb, y_ps)
                                nc.sync.dma_start(yT_r[n, :, sl_prev], y_sb)
                    out_prev = out_blk
                    sl_prev = sl

                # tail: Wo for the last lb
                for n in range(KC):
                    y_ps = uyp.tile([P, LB], f32, tag="yps")
                    for dt_ in range(DGT):
                        nc.tensor.matmul(
                            y_ps,
                            (wo_sb[:, dt_, n * P:(n + 1) * P]),
                            (out_prev[:, dt_]),
                            start=(dt_ == 0), stop=(dt_ == DGT - 1),
                        )
                    y_sb = ysp.tile([P, LB], bf16, tag="ysb")
                    if n % 2 == 0:
                        nc.scalar.copy(y_sb, y_ps)
                    else:
                        nc.vector.tensor_copy(y_sb, y_ps)
                    nc.sync.dma_start(yT_r[n, :, sl_prev], y_sb)
    return nc


_NC_CACHE = {}


def _get_nc():
    if "nc" not in _NC_CACHE:
        nc = bacc.Bacc("TRN2", debug=False, num_devices=NCORES)
        build_kernel(nc)
        nc.finalize()  # runs Bacc.compile(): wait splitting + reg alloc
        _NC_CACHE["nc"] = nc
    return _NC_CACHE["nc"]


def make_in_maps(x, Wq, Wk, Wv, Wo, proj_k, proj_v):
    f = np.float32
    x = np.asarray(x, f)
    Wq = np.asarray(Wq, f)
    Wk = np.asarray(Wk, f)
    Wv = np.asarray(Wv, f)
    Wo = np.asarray(Wo, f)
    pkv = np.ascontiguousarray(
        np.concatenate([np.asarray(proj_k, f), np.asarray(proj_v, f)], axis=1))
    pkv_b = _to_bf16(pkv)
    in_maps = []
    for c in range(NCORES):
        b, hg = divmod(c, HG)
        cs = slice(hg * DG, (hg + 1) * DG)
        in_maps.append({
            "xT": _to_bf16(x[b].T),
            "xn": _to_bf16(x[b]),
            "wq": _to_bf16(Wq[:, cs] * SCALE),
            "wk": _to_bf16(Wk[:, cs]),
            "wv": _to_bf16(Wv[:, cs]),
            "wo": _to_bf16(Wo[cs, :]),
            "pkv": pkv_b,
        })
    return in_maps


def _to_bf16(a):
    import ml_dtypes
    return np.ascontiguousarray(np.asarray(a, np.float32)).astype(ml_dtypes.bfloat16)


def gather_output(results):
    outs = [np.asarray(results[c]["yT"], np.float32) for c in range(NCORES)]
    y = np.stack([(outs[HG * b] + outs[HG * b + 1]).T for b in range(B)])
    return np.ascontiguousarray(y, np.float32)


def kernel(x, Wq, Wk, Wv, Wo, proj_k, proj_v, _trace=False, _trace_kwargs=None):
    nc = _get_nc()
    in_maps = make_in_maps(x, Wq, Wk, Wv, Wo, proj_k, proj_v)
    res = run_bass_kernel_spmd(
        nc, in_maps, core_ids=list(range(NCORES)),
        trace=_trace, **(_trace_kwargs or {}),
    )
    out = gather_output(res.results)
    if _trace:
        return out, res
    return out


# revision 38
# speedup vs baseline: 1.0211x; 1.0211x over previous
# Linformer attention kernel for Trainium2 (8 NeuronCores, SPMD, no collectives).
#
# Sharding: core c = (batch b = c//2, head-group hg = c%2). Each core runs one
# batch's Linformer attention for 8 of the 16 heads: Wq/Wk/Wv column-sliced
# (512 cols), Wo row-sliced -> partial [D, L] output (transposed); host sums
# the two partials per batch and transposes back.
#
# Key algebraic restructure: the Linformer low-rank projection commutes with
# the K/V linears: k_proj = Wk^T (x^T pk). So we never materialize full K/V:
# compute xkv^T = x^T [pk || pv] then two small [K, D] GEMMs instead of two
# full [L, DG] GEMMs + reduces.
#
# v5 vs the original baseline: xn/pkv tile DMAs lead each lb and the xkv
# matmuls are emitted before the qT matmuls, so the PE starts ~2us into the
# kernel instead of ~30us; wq loads right after lb0's tiles and wk/wv/wo
# stream in small chunks across lbs 1..6 (they are needed only at the
# epilogue); the qT path (xT, wq) runs in bf16, halving the biggest input
# stream; the yT output is bf16, halving the output stream (host sums the
# two partials in f32).

import os
import sys

for _p in ("/opt/trn_rl_repo", "/root/.axon_site/_ro/trn_rl_repo"):
    if os.path.isdir(_p) and _p not in sys.path:
        sys.path.insert(0, _p)
        break

import numpy as np

import concourse.bass as bass
from concourse import bacc
import concourse.mybir as mybir
from concourse.bass_utils import run_bass_kernel_spmd
from concourse.tile import TileContext

P = 128
B, L, D = 4, 4096, 1024
H, HD = 16, 64
K = 256
SCALE = HD ** -0.5

NCORES = 8
HG = 2                 # head groups (cores per batch)
DG = D // HG           # 512: local width of Wq/Wk/Wv cols & Wo rows
HL = H // HG           # 8 local heads
KC = D // P            # 8 contraction chunks over D
DGT = DG // P          # 4 partition-tiles over local d
KT = K // P            # 2 partition-tiles over low-rank K
K2 = 2 * K             # 512: pk || pv concat width
LB = 512               # L block
NLB = L // LB          # 8

f32 = mybir.dt.float32
f32r = mybir.dt.float32r
bf16 = mybir.dt.bfloat16
AF = mybir.ActivationFunctionType


def build_kernel(nc: bass.Bass):
    xT = nc.dram_tensor("xT", (D, L), bf16, kind="ExternalInput")
    xn = nc.dram_tensor("xn", (L, D), bf16, kind="ExternalInput")
    wq = nc.dram_tensor("wq", (D, DG), bf16, kind="ExternalInput")
    wk = nc.dram_tensor("wk", (D, DG), bf16, kind="ExternalInput")
    wv = nc.dram_tensor("wv", (D, DG), bf16, kind="ExternalInput")
    wo = nc.dram_tensor("wo", (DG, D), bf16, kind="ExternalInput")
    pkv = nc.dram_tensor("pkv", (L, K2), bf16, kind="ExternalInput")
    yT = nc.dram_tensor("yT", (D, L), bf16, kind="ExternalOutput")

    xn_r = xn.rearrange("(lt p) c -> lt p c", p=P)      # [32,128,1024]
    pkv_r = pkv.rearrange("(lt p) k -> lt p k", p=P)    # [32,128,512]
    yT_r = yT.rearrange("(nt p) l -> nt p l", p=P)      # [8,128,4096]
    xT_r = xT.rearrange("(kc p) l -> p kc l", p=P)      # [128,8,4096]
    wk_r = wk.rearrange("(kc p) m -> p kc m", p=P)
    wv_r = wv.rearrange("(kc p) m -> p kc m", p=P)
    wo_r = wo.rearrange("(dt p) n -> p dt n", p=P)

    with TileContext(nc) as tc:
        with tc.tile_pool(name="const", bufs=1) as cpool:
            # Resident tensors (per-partition bytes in comments).
            wq_sb = cpool.tile([P, KC, DG], bf16)        # 8K
            wk_sb = cpool.tile([P, KC, DG], bf16)        # 8K
            wv_sb = cpool.tile([P, KC, DG], bf16)        # 8K
            wo_sb = cpool.tile([P, DGT, D], bf16)        # 8K
            qT_sb = cpool.tile([P, DGT, L], bf16)        # 32K
            xkvT_sb = cpool.tile([P, KC, K2], f32r)      # 16K  x^T [pk||pv]
            xkv_bf = cpool.tile([P, KC, K2], bf16)       # 8K   bf16 copy
            kpT_pad = cpool.tile([P, HL, K], bf16)       # 4K   zero-padded/head
            vpa_sb = cpool.tile([P, KT, HL, P], bf16)    # 4K per-head slabs
            ones_c = cpool.tile([P, 1], bf16)            # denom stationary col
            scr = cpool.tile([P, 16], f32)               # gpsimd lib warmup
            nc.vector.memset(kpT_pad[:].bitcast(bf16), 0.0)
            nc.vector.memset(vpa_sb[:].bitcast(bf16), 0.0)
            nc.vector.memset(ones_c[:].bitcast(bf16), 1.0)
            nc.vector.memset(scr[:].bitcast(f32), 1.0)
            # dummy broadcast: forces the gpsimd ucode library load at t=0
            # (otherwise LOAD_LIB stalls the first real broadcast ~7us at
            # the phase-1 -> phase-2 transition)
            nc.gpsimd.partition_broadcast(scr[:, :], scr[0:1, :])
            # odd heads (values@p64..127): ones col 0 -> denom@p0 for free.
            # Even heads (values@p0..63) get a separate denominator matmul on
            # the PE: its partition-0 output feeds the reciprocal directly
            # (PE has slack in phase 2; DVE is the binding engine there, and
            # the ucode recip/broadcast only accept partition-0 operands).
            for kpt in range(KT):
                for h in range(1, HL, 2):
                    nc.vector.memset(
                        vpa_sb[:, kpt, h, 0:1].bitcast(bf16), 1.0)

            # ---------------- Phase 1: xkvT accumulation + qT ---------------
            with tc.tile_pool(name="p1_mm", bufs=2, space="PSUM") as mmp, \
                 tc.tile_pool(name="p1_xk", bufs=2, space="PSUM") as xkp, \
                 tc.tile_pool(name="p1_x", bufs=2) as xsp, \
                 tc.tile_pool(name="p1_xn", bufs=5) as xnp, \
                 tc.tile_pool(name="p1_p", bufs=5) as psp:
                for lb in range(NLB):
                    sl = slice(lb * LB, (lb + 1) * LB)

                    # DMA order: xn/pkv first (xkv matmuls start ~2us in),
                    # then wq + the xT block for the qT matmuls.
                    xns = []
                    pkvs = []
                    for j in range(LB // P):
                        lt = lb * (LB // P) + j
                        xnt = xnp.tile([P, D], bf16, tag="xn")
                        nc.sync.dma_start(xnt, xn_r[lt])
                        xns.append(xnt)
                        pkvt = psp.tile([P, K2], bf16, tag="pkv")
                        nc.sync.dma_start(pkvt, pkv_r[lt])
                        pkvs.append(pkvt)
                    if lb == 0:
                        nc.sync.dma_start(
                            wq_sb[:], wq.rearrange("(kc p) m -> p kc m", p=P))
                    xTb = xsp.tile([P, KC, LB], bf16, tag="xTb")
                    for kc2 in range(0, KC, 4):
                        nc.sync.dma_start(
                            xTb[:, kc2:kc2 + 4], xT_r[:, kc2:kc2 + 4, sl])
                    # wk/wv/wo are needed only at the epilogue: stream them
                    # in small chunks across lbs 1..6 so they never
                    # head-block the x/pkv tile stream.
                    if 1 <= lb <= 6:
                        wchunks = [(wk_sb, wk_r, 0, 4), (wk_sb, wk_r, 4, 8),
                                   (wv_sb, wv_r, 0, 4), (wv_sb, wv_r, 4, 8),
                                   (wo_sb, wo_r, 0, 2), (wo_sb, wo_r, 2, 4)]
                        dst, src, c0, c1 = wchunks[lb - 1]
                        for c in range(c0, c1):
                            nc.sync.dma_start(dst[:, c:c + 1], src[:, c:c + 1])

                    # xkvT[c, :] += x[l-blk, c]^T @ [pk || pv][l-blk, :]
                    for cc in range(KC):
                        xk_ps = xkp.tile([P, K2], f32, tag="xk")
                        for j in range(LB // P):
                            nc.tensor.matmul(
                                xk_ps,
                                (xns[j][:, cc * P:(cc + 1) * P]),
                                (pkvs[j]),
                                start=(j == 0), stop=(j == LB // P - 1),
                            )
                        if lb == 0:
                            nc.vector.tensor_copy(xkvT_sb[:, cc], xk_ps)
                        elif lb < NLB - 1:
                            nc.vector.tensor_add(
                                xkvT_sb[:, cc],
                                xkvT_sb[:, cc].bitcast(f32), xk_ps)
                        else:
                            # last add writes the bf16 copy directly (kp/vp
                            # matmuls consume bf16 operands)
                            nc.vector.tensor_add(
                                xkv_bf[:, cc],
                                xkvT_sb[:, cc].bitcast(f32), xk_ps)

                    # kp/vp epilogue runs BEFORE the last lb's qT matmuls so
                    # its DVE copies and the phase-2 pool handoff hide under
                    # the qT chains instead of stalling the PE.
                    if lb == NLB - 1:
                        # kpT[dt] = Wk-chunk^T @ xkT -> zero-padded per head
                        for dt_ in range(DGT):
                            kp_ps = mmp.tile([P, K], f32, tag="kp")
                            for cc in range(KC):
                                nc.tensor.matmul(
                                    kp_ps,
                                    (wk_sb[:, cc, dt_ * P:(dt_ + 1) * P]),
                                    (xkv_bf[:, cc, 0:K]),
                                    start=(cc == 0), stop=(cc == KC - 1),
                                )
                            for hh in range(2):
                                h = dt_ * 2 + hh
                                off = hh * 64
                                nc.vector.tensor_copy(
                                    kpT_pad[off:off + 64, h, :],
                                    kp_ps[off:off + 64])

                        # v_proj[k-half] = xvT^T @ Wv -> per-head padded
                        for kh in range(KT):
                            vp_ps = mmp.tile([P, DG], f32, tag="vp")
                            for cc in range(KC):
                                nc.tensor.matmul(
                                    vp_ps,
                                    (xkv_bf[:, cc, K + kh * P:K + (kh + 1) * P]),
                                    (wv_sb[:, cc]),
                                    start=(cc == 0), stop=(cc == KC - 1),
                                )
                            for h in range(HL):
                                off = (h % 2) * 64
                                nc.vector.tensor_copy(
                                    vpa_sb[:, kh, h, off:off + HD],
                                    vp_ps[:, h * HD:(h + 1) * HD],
                                )

                    # qT[dpt, lb-block] = (Wq columns)^T @ x^T-block
                    for dt_ in range(DGT):
                        q_ps = mmp.tile([P, LB], f32, tag="mm")
                        for kc in range(KC):
                            nc.tensor.matmul(
                                q_ps,
                                (wq_sb[:, kc, dt_ * P:(dt_ + 1) * P]),
                                (xTb[:, kc]),
                                start=(kc == 0), stop=(kc == KC - 1),
                            )
                        nc.scalar.copy(qT_sb[:, dt_, sl], q_ps)

            # ---------------- Phase 2+3: attention + output projection ------
            with tc.tile_pool(name="p2_lg", bufs=2, space="PSUM") as lgp, \
                 tc.tile_pool(name="p2_av", bufs=3, space="PSUM") as avp, \
                 tc.tile_pool(name="p2_dn", bufs=1, space="PSUM") as dnp, \
                 tc.tile_pool(name="p2_y", bufs=2, space="PSUM") as uyp, \
                 tc.tile_pool(name="p2_e", bufs=4) as ep2, \
                 tc.tile_pool(name="p2_o", bufs=2) as op2, \
                 tc.tile_pool(name="p2_r", bufs=4) as rp2, \
                 tc.tile_pool(name="p2_ys", bufs=4) as ysp:
                out_prev = None
                sl_prev = None
                for lb in range(NLB):
                    sl = slice(lb * LB, (lb + 1) * LB)
                    out_blk = op2.tile([P, DGT, LB], bf16, tag="oblk")

                    # software-pipelined: logits(h) issued on PE before
                    # attn@v(h-1); Wo chains of the PREVIOUS lb are threaded
                    # through this head loop (one n-chunk per head) so the
                    # normalization tail of head h never stalls the PE.
                    e_tiles = {}
                    for h in range(HL + 1):
                        if h < HL:
                            dt_ = h // 2
                            es = []
                            for kpt in range(KT):
                                lg_ps = lgp.tile([P, LB], f32, tag="lg")
                                nc.tensor.matmul(
                                    lg_ps,
                                    (kpT_pad[:, h, kpt * P:(kpt + 1) * P]),
                                    (qT_sb[:, dt_, sl]),
                                    start=True, stop=True,
                                )
                                e_sb = ep2.tile([P, LB], bf16, tag="eT")
                                nc.scalar.activation(e_sb, lg_ps, AF.Exp)
                                es.append(e_sb)
                            e_tiles[h] = es
                        if h > 0:
                            hp = h - 1
                            dtp = hp // 2
                            off = (hp % 2) * 64
                            es = e_tiles.pop(hp)
                            av_ps = avp.tile([P, LB], f32, tag="av")
                            for kpt in range(KT):
                                nc.tensor.matmul(
                                    av_ps,
                                    (vpa_sb[:, kpt, hp]),
                                    (es[kpt]),
                                    start=(kpt == 0), stop=(kpt == KT - 1),
                                )
                            if off == 0:
                                # even head: values@p0..63; denom via a
                                # separate PE matmul (p0 output, ucode-safe)
                                dn_ps = dnp.tile([1, LB], f32, tag="dn")
                                for kpt in range(KT):
                                    nc.tensor.matmul(
                                        dn_ps,
                                        (ones_c),
                                        (es[kpt]),
                                        start=(kpt == 0), stop=(kpt == KT - 1),
                                    )
                                den_ap = dn_ps[0:1]
                            else:
                                # odd head: ones col in vpa -> denom@p0
                                den_ap = av_ps[0:1]
                            rT2 = rp2.tile([P, LB], f32, tag="r2")
                            nc.vector.reciprocal_approx_fast(
                                rT2[0:1, :], den_ap)
                            rb = rp2.tile([P, LB], f32, tag="rb")
                            nc.gpsimd.partition_broadcast(rb[:, :], rT2[0:1, :])
                            nc.vector.tensor_tensor(
                                out_blk[off:off + 64, dtp],
                                av_ps[off:off + 64],
                                rb[off:off + 64],
                                mybir.AluOpType.mult,
                            )
                        # interleave one Wo chain of the prev lb; shifted one
                        # slot late (n = h-1) so the first chain doesn't wait
                        # on the tail of out_prev's normalization
                        if out_prev is not None:
                            n = h - 1 if h >= 1 else None
                            if n is not None:
                                y_ps = uyp.tile([P, LB], f32, tag="yps")
                                for dt_ in range(DGT):
                                    nc.tensor.matmul(
                                        y_ps,
                                        (wo_sb[:, dt_, n * P:(n + 1) * P]),
                                        (out_prev[:, dt_]),
                                        start=(dt_ == 0), stop=(dt_ == DGT - 1),
                                    )
                                y_sb = ysp.tile([P, LB], bf16, tag="ysb")
                                if n % 2 == 0:
                                    nc.scalar.copy(y_sb, y_ps)
                                else:
                                    nc.vector.tensor_copy(y_sb, y_ps)
                                nc.sync.dma_start(yT_r[n, :, sl_prev], y_sb)
                    out_prev = out_blk
                    sl_prev = sl

                # tail: Wo for the last lb
                for n in range(KC):
                    y_ps = uyp.tile([P, LB], f32, tag="yps")
                    for dt_ in range(DGT):
                        nc.tensor.matmul(
                            y_ps,
                            (wo_sb[:, dt_, n * P:(n + 1) * P]),
                            (out_prev[:, dt_]),
                            start=(dt_ == 0), stop=(dt_ == DGT - 1),
                        )
                    y_sb = ysp.tile([P, LB], bf16, tag="ysb")
                    if n % 2 == 0:
                        nc.scalar.copy(y_sb, y_ps)
                    else:
                        nc.vector.tensor_copy(y_sb, y_ps)
                    nc.sync.dma_start(yT_r[n, :, sl_prev], y_sb)
    return nc


_NC_CACHE = {}


def _get_nc():
    if "nc" not in _NC_CACHE:
        nc = bacc.Bacc("TRN2", debug=False, num_devices=NCORES)
        build_kernel(nc)
        nc.finalize()  # runs Bacc.compile(): wait splitting + reg alloc
        _NC_CACHE["nc"] = nc
    return _NC_CACHE["nc"]


def make_in_maps(x, Wq, Wk, Wv, Wo, proj_k, proj_v):
    f = np.float32
    x = np.asarray(x, f)
    Wq = np.asarray(Wq, f)
    Wk = np.asarray(Wk, f)
    Wv = np.asarray(Wv, f)
    Wo = np.asarray(Wo, f)
    pkv = np.ascontiguousarray(
        np.concatenate([np.asarray(proj_k, f), np.asarray(proj_v, f)], axis=1))
    pkv_b = _to_bf16(pkv)
    in_maps = []
    for c in range(NCORES):
        b, hg = divmod(c, HG)
        cs = slice(hg * DG, (hg + 1) * DG)
        in_maps.append({
            "xT": _to_bf16(x[b].T),
            "xn": _to_bf16(x[b]),
            "wq": _to_bf16(Wq[:, cs] * SCALE),
            "wk": _to_bf16(Wk[:, cs]),
            "wv": _to_bf16(Wv[:, cs]),
            "wo": _to_bf16(Wo[cs, :]),
            "pkv": pkv_b,
        })
    return in_maps


def _to_bf16(a):
    import ml_dtypes
    return np.ascontiguousarray(np.asarray(a, np.float32)).astype(ml_dtypes.bfloat16)


def gather_output(results):
    outs = [np.asarray(results[c]["yT"], np.float32) for c in range(NCORES)]
    y = np.stack([(outs[HG * b] + outs[HG * b + 1]).T for b in range(B)])
    return np.ascontiguousarray(y, np.float32)


def kernel(x, Wq, Wk, Wv, Wo, proj_k, proj_v, _trace=False, _trace_kwargs=None):
    nc = _get_nc()
    in_maps = make_in_maps(x, Wq, Wk, Wv, Wo, proj_k, proj_v)
    res = run_bass_kernel_spmd(
        nc, in_maps, core_ids=list(range(NCORES)),
        trace=_trace, **(_trace_kwargs or {}),
    )
    out = gather_output(res.results)
    if _trace:
        return out, res
    return out


# revision 40
# speedup vs baseline: 1.0352x; 1.0137x over previous
# Linformer attention kernel for Trainium2 (8 NeuronCores, SPMD, no collectives).
#
# Sharding: core c = (batch b = c//2, head-group hg = c%2). Each core runs one
# batch's Linformer attention for 8 of the 16 heads: Wq/Wk/Wv column-sliced
# (512 cols), Wo row-sliced -> partial [D, L] output (transposed); host sums
# the two partials per batch and transposes back.
#
# Key algebraic restructure: the Linformer low-rank projection commutes with
# the K/V linears: k_proj = Wk^T (x^T pk). So we never materialize full K/V:
# compute xkv^T = x^T [pk || pv] then two small [K, D] GEMMs instead of two
# full [L, DG] GEMMs + reduces.
#
# v5 vs the original baseline: xn/pkv tile DMAs lead each lb and the xkv
# matmuls are emitted before the qT matmuls, so the PE starts ~2us into the
# kernel instead of ~30us; wq loads right after lb0's tiles and wk/wv/wo
# stream in small chunks across lbs 1..6 (they are needed only at the
# epilogue); the qT path (xT, wq) runs in bf16, halving the biggest input
# stream; the yT output is bf16, halving the output stream (host sums the
# two partials in f32).

import os
import sys

for _p in ("/opt/trn_rl_repo", "/root/.axon_site/_ro/trn_rl_repo"):
    if os.path.isdir(_p) and _p not in sys.path:
        sys.path.insert(0, _p)
        break

import numpy as np

import concourse.bass as bass
from concourse import bacc
import concourse.mybir as mybir
from concourse.bass_utils import run_bass_kernel_spmd
from concourse.tile import TileContext

P = 128
B, L, D = 4, 4096, 1024
H, HD = 16, 64
K = 256
SCALE = HD ** -0.5

NCORES = 8
HG = 2                 # head groups (cores per batch)
DG = D // HG           # 512: local width of Wq/Wk/Wv cols & Wo rows
HL = H // HG           # 8 local heads
KC = D // P            # 8 contraction chunks over D
DGT = DG // P          # 4 partition-tiles over local d
KT = K // P            # 2 partition-tiles over low-rank K
K2 = 2 * K             # 512: pk || pv concat width
LB = 512               # L block
NLB = L // LB          # 8

f32 = mybir.dt.float32
f32r = mybir.dt.float32r
bf16 = mybir.dt.bfloat16
AF = mybir.ActivationFunctionType


def build_kernel(nc: bass.Bass):
    xT = nc.dram_tensor("xT", (D, L), bf16, kind="ExternalInput")
    xn = nc.dram_tensor("xn", (L, D), bf16, kind="ExternalInput")
    wq = nc.dram_tensor("wq", (D, DG), bf16, kind="ExternalInput")
    wk = nc.dram_tensor("wk", (D, DG), bf16, kind="ExternalInput")
    wv = nc.dram_tensor("wv", (D, DG), bf16, kind="ExternalInput")
    wo = nc.dram_tensor("wo", (DG, D), bf16, kind="ExternalInput")
    pkv = nc.dram_tensor("pkv", (L, K2), bf16, kind="ExternalInput")
    yT = nc.dram_tensor("yT", (D, L), bf16, kind="ExternalOutput")

    xn_r = xn.rearrange("(lt p) c -> lt p c", p=P)      # [32,128,1024]
    pkv_r = pkv.rearrange("(lt p) k -> lt p k", p=P)    # [32,128,512]
    yT_r = yT.rearrange("(nt p) l -> nt p l", p=P)      # [8,128,4096]
    xT_r = xT.rearrange("(kc p) l -> p kc l", p=P)      # [128,8,4096]
    wk_r = wk.rearrange("(kc p) m -> p kc m", p=P)
    wv_r = wv.rearrange("(kc p) m -> p kc m", p=P)
    wo_r = wo.rearrange("(dt p) n -> p dt n", p=P)

    with TileContext(nc) as tc:
        with tc.tile_pool(name="const", bufs=1) as cpool:
            # Resident tensors (per-partition bytes in comments).
            wq_sb = cpool.tile([P, KC, DG], bf16)        # 8K
            wk_sb = cpool.tile([P, KC, DG], bf16)        # 8K
            wv_sb = cpool.tile([P, KC, DG], bf16)        # 8K
            wo_sb = cpool.tile([P, DGT, D], bf16)        # 8K
            qT_sb = cpool.tile([P, DGT, L], bf16)        # 32K
            xkvT_sb = cpool.tile([P, KC, K2], f32r)      # 16K  x^T [pk||pv]
            xkv_bf = cpool.tile([P, KC, K2], bf16)       # 8K   bf16 copy
            kpT_pad = cpool.tile([P, HL, K], bf16)       # 4K   zero-padded/head
            vpa_sb = cpool.tile([P, KT, HL, P], bf16)    # 4K per-head slabs
            ones_c = cpool.tile([P, 1], bf16)            # denom stationary col
            scr = cpool.tile([P, 16], f32)               # gpsimd lib warmup
            nc.vector.memset(kpT_pad[:].bitcast(bf16), 0.0)
            nc.vector.memset(vpa_sb[:].bitcast(bf16), 0.0)
            nc.vector.memset(ones_c[:].bitcast(bf16), 1.0)
            nc.vector.memset(scr[:].bitcast(f32), 1.0)
            # dummy broadcast: forces the gpsimd ucode library load at t=0
            # (otherwise LOAD_LIB stalls the first real broadcast ~7us at
            # the phase-1 -> phase-2 transition)
            nc.gpsimd.partition_broadcast(scr[:, :], scr[0:1, :])
            # odd heads (values@p64..127): ones col 0 -> denom@p0 for free.
            # Even heads (values@p0..63) get a separate denominator matmul on
            # the PE: its partition-0 output feeds the reciprocal directly
            # (PE has slack in phase 2; DVE is the binding engine there, and
            # the ucode recip/broadcast only accept partition-0 operands).
            for kpt in range(KT):
                for h in range(1, HL, 2):
                    nc.vector.memset(
                        vpa_sb[:, kpt, h, 0:1].bitcast(bf16), 1.0)

            # ---------------- Phase 1: xkvT accumulation + qT ---------------
            with tc.tile_pool(name="p1_mm", bufs=2, space="PSUM") as mmp, \
                 tc.tile_pool(name="p1_xk", bufs=2, space="PSUM") as xkp, \
                 tc.tile_pool(name="p1_x", bufs=3) as xsp, \
                 tc.tile_pool(name="p1_xn", bufs=9) as xnp, \
                 tc.tile_pool(name="p1_p", bufs=9) as psp:
                for lb in range(NLB):
                    sl = slice(lb * LB, (lb + 1) * LB)

                    # DMA order: xn/pkv first (xkv matmuls start ~2us in),
                    # then wq + the xT block for the qT matmuls.
                    xns = []
                    pkvs = []
                    for j in range(LB // P):
                        lt = lb * (LB // P) + j
                        xnt = xnp.tile([P, D], bf16, tag="xn")
                        nc.sync.dma_start(xnt, xn_r[lt])
                        xns.append(xnt)
                        pkvt = psp.tile([P, K2], bf16, tag="pkv")
                        nc.sync.dma_start(pkvt, pkv_r[lt])
                        pkvs.append(pkvt)
                    if lb == 0:
                        nc.sync.dma_start(
                            wq_sb[:], wq.rearrange("(kc p) m -> p kc m", p=P))
                    xTb = xsp.tile([P, KC, LB], bf16, tag="xTb")
                    for kc2 in range(0, KC, 4):
                        nc.sync.dma_start(
                            xTb[:, kc2:kc2 + 4], xT_r[:, kc2:kc2 + 4, sl])
                    # wk/wv/wo are needed only at the epilogue: stream them
                    # in small chunks across lbs 1..6 so they never
                    # head-block the x/pkv tile stream.
                    if 1 <= lb <= 6:
                        wchunks = [(wk_sb, wk_r, 0, 4), (wk_sb, wk_r, 4, 8),
                                   (wv_sb, wv_r, 0, 4), (wv_sb, wv_r, 4, 8),
                                   (wo_sb, wo_r, 0, 2), (wo_sb, wo_r, 2, 4)]
                        dst, src, c0, c1 = wchunks[lb - 1]
                        for c in range(c0, c1):
                            nc.sync.dma_start(dst[:, c:c + 1], src[:, c:c + 1])

                    # xkvT[c, :] += x[l-blk, c]^T @ [pk || pv][l-blk, :]
                    for cc in range(KC):
                        xk_ps = xkp.tile([P, K2], f32, tag="xk")
                        for j in range(LB // P):
                            nc.tensor.matmul(
                                xk_ps,
                                (xns[j][:, cc * P:(cc + 1) * P]),
                                (pkvs[j]),
                                start=(j == 0), stop=(j == LB // P - 1),
                            )
                        if lb == 0:
                            nc.vector.tensor_copy(xkvT_sb[:, cc], xk_ps)
                        elif lb < NLB - 1:
                            nc.vector.tensor_add(
                                xkvT_sb[:, cc],
                                xkvT_sb[:, cc].bitcast(f32), xk_ps)
                        else:
                            # last add writes the bf16 copy directly (kp/vp
                            # matmuls consume bf16 operands)
                            nc.vector.tensor_add(
                                xkv_bf[:, cc],
                                xkvT_sb[:, cc].bitcast(f32), xk_ps)

                    # kp/vp epilogue runs BEFORE the last lb's qT matmuls so
                    # its DVE copies and the phase-2 pool handoff hide under
                    # the qT chains instead of stalling the PE.
                    if lb == NLB - 1:
                        # kpT[dt] = Wk-chunk^T @ xkT -> zero-padded per head
                        for dt_ in range(DGT):
                            kp_ps = mmp.tile([P, K], f32, tag="kp")
                            for cc in range(KC):
                                nc.tensor.matmul(
                                    kp_ps,
                                    (wk_sb[:, cc, dt_ * P:(dt_ + 1) * P]),
                                    (xkv_bf[:, cc, 0:K]),
                                    start=(cc == 0), stop=(cc == KC - 1),
                                )
                            for hh in range(2):
                                h = dt_ * 2 + hh
                                off = hh * 64
                                nc.vector.tensor_copy(
                                    kpT_pad[off:off + 64, h, :],
                                    kp_ps[off:off + 64])

                        # v_proj[k-half] = xvT^T @ Wv -> per-head padded
                        for kh in range(KT):
                            vp_ps = mmp.tile([P, DG], f32, tag="vp")
                            for cc in range(KC):
                                nc.tensor.matmul(
                                    vp_ps,
                                    (xkv_bf[:, cc, K + kh * P:K + (kh + 1) * P]),
                                    (wv_sb[:, cc]),
                                    start=(cc == 0), stop=(cc == KC - 1),
                                )
                            for h in range(HL):
                                off = (h % 2) * 64
                                nc.vector.tensor_copy(
                                    vpa_sb[:, kh, h, off:off + HD],
                                    vp_ps[:, h * HD:(h + 1) * HD],
                                )

                    # qT[dpt, lb-block] = (Wq columns)^T @ x^T-block
                    for dt_ in range(DGT):
                        q_ps = mmp.tile([P, LB], f32, tag="mm")
                        for kc in range(KC):
                            nc.tensor.matmul(
                                q_ps,
                                (wq_sb[:, kc, dt_ * P:(dt_ + 1) * P]),
                                (xTb[:, kc]),
                                start=(kc == 0), stop=(kc == KC - 1),
                            )
                        nc.scalar.copy(qT_sb[:, dt_, sl], q_ps)

            # ---------------- Phase 2+3: attention + output projection ------
            with tc.tile_pool(name="p2_lg", bufs=2, space="PSUM") as lgp, \
                 tc.tile_pool(name="p2_av", bufs=3, space="PSUM") as avp, \
                 tc.tile_pool(name="p2_dn", bufs=1, space="PSUM") as dnp, \
                 tc.tile_pool(name="p2_y", bufs=2, space="PSUM") as uyp, \
                 tc.tile_pool(name="p2_e", bufs=6) as ep2, \
                 tc.tile_pool(name="p2_o", bufs=3) as op2, \
                 tc.tile_pool(name="p2_r", bufs=4) as rp2, \
                 tc.tile_pool(name="p2_ys", bufs=6) as ysp:
                out_prev = None
                sl_prev = None
                for lb in range(NLB):
                    sl = slice(lb * LB, (lb + 1) * LB)
                    out_blk = op2.tile([P, DGT, LB], bf16, tag="oblk")

                    # software-pipelined: logits(h) issued on PE before
                    # attn@v(h-1); Wo chains of the PREVIOUS lb are threaded
                    # through this head loop (one n-chunk per head) so the
                    # normalization tail of head h never stalls the PE.
                    e_tiles = {}
                    for h in range(HL + 1):
                        if h < HL:
                            dt_ = h // 2
                            es = []
                            for kpt in range(KT):
                                lg_ps = lgp.tile([P, LB], f32, tag="lg")
                                nc.tensor.matmul(
                                    lg_ps,
                                    (kpT_pad[:, h, kpt * P:(kpt + 1) * P]),
                                    (qT_sb[:, dt_, sl]),
                                    start=True, stop=True,
                                )
                                e_sb = ep2.tile([P, LB], bf16, tag="eT")
                                nc.scalar.activation(e_sb, lg_ps, AF.Exp)
                                es.append(e_sb)
                            e_tiles[h] = es
                        if h > 0:
                            hp = h - 1
                            dtp = hp // 2
                            off = (hp % 2) * 64
                            es = e_tiles.pop(hp)
                            av_ps = avp.tile([P, LB], f32, tag="av")
                            for kpt in range(KT):
                                nc.tensor.matmul(
                                    av_ps,
                                    (vpa_sb[:, kpt, hp]),
                                    (es[kpt]),
                                    start=(kpt == 0), stop=(kpt == KT - 1),
                                )
                            if off == 0:
                                # even head: values@p0..63; denom via a
                                # separate PE matmul (p0 output, ucode-safe)
                                dn_ps = dnp.tile([1, LB], f32, tag="dn")
                                for kpt in range(KT):
                                    nc.tensor.matmul(
                                        dn_ps,
                                        (ones_c),
                                        (es[kpt]),
                                        start=(kpt == 0), stop=(kpt == KT - 1),
                                    )
                                den_ap = dn_ps[0:1]
                            else:
                                # odd head: ones col in vpa -> denom@p0
                                den_ap = av_ps[0:1]
                            rT2 = rp2.tile([P, LB], f32, tag="r2")
                            nc.vector.reciprocal_approx_fast(
                                rT2[0:1, :], den_ap)
                            rb = rp2.tile([P, LB], f32, tag="rb")
                            nc.gpsimd.partition_broadcast(rb[:, :], rT2[0:1, :])
                            nc.vector.tensor_tensor(
                                out_blk[off:off + 64, dtp],
                                av_ps[off:off + 64],
                                rb[off:off + 64],
                                mybir.AluOpType.mult,
                            )
                        # interleave one Wo chain of the prev lb; shifted one
                        # slot late (n = h-1) so the first chain doesn't wait
                        # on the tail of out_prev's normalization
                        if out_prev is not None:
                            n = h - 1 if h >= 1 else None
                            if n is not None:
                                y_ps = uyp.tile([P, LB], f32, tag="yps")
                                for dt_ in range(DGT):
                                    nc.tensor.matmul(
                                        y_ps,
                                        (wo_sb[:, dt_, n * P:(n + 1) * P]),
                                        (out_prev[:, dt_]),
                                        start=(dt_ == 0), stop=(dt_ == DGT - 1),
                                    )
                                y_sb = ysp.tile([P, LB], bf16, tag="ysb")
                                if n % 2 == 0:
                                    nc.scalar.copy(y_sb, y_ps)
                                else:
                                    nc.vector.tensor_copy(y_sb, y_ps)
                                nc.sync.dma_start(yT_r[n, :, sl_prev], y_sb)
                    out_prev = out_blk
                    sl_prev = sl

                # tail: Wo for the last lb
                for n in range(KC):
                    y_ps = uyp.tile([P, LB], f32, tag="yps")
                    for dt_ in range(DGT):
                        nc.tensor.matmul(
                            y_ps,
                            (wo_sb[:, dt_, n * P:(n + 1) * P]),
                            (out_prev[:, dt_]),
                            start=(dt_ == 0), stop=(dt_ == DGT - 1),
                        )
                    y_sb = ysp.tile([P, LB], bf16, tag="ysb")
                    if n % 2 == 0:
                        nc.scalar.copy(y_sb, y_ps)
                    else:
                        nc.vector.tensor_copy(y_sb, y_ps)
                    nc.sync.dma_start(yT_r[n, :, sl_prev], y_sb)
    return nc


_NC_CACHE = {}


def _get_nc():
    if "nc" not in _NC_CACHE:
        nc = bacc.Bacc("TRN2", debug=False, num_devices=NCORES)
        build_kernel(nc)
        nc.finalize()  # runs Bacc.compile(): wait splitting + reg alloc
        _NC_CACHE["nc"] = nc
    return _NC_CACHE["nc"]


def make_in_maps(x, Wq, Wk, Wv, Wo, proj_k, proj_v):
    f = np.float32
    x = np.asarray(x, f)
    Wq = np.asarray(Wq, f)
    Wk = np.asarray(Wk, f)
    Wv = np.asarray(Wv, f)
    Wo = np.asarray(Wo, f)
    pkv = np.ascontiguousarray(
        np.concatenate([np.asarray(proj_k, f), np.asarray(proj_v, f)], axis=1))
    pkv_b = _to_bf16(pkv)
    in_maps = []
    for c in range(NCORES):
        b, hg = divmod(c, HG)
        cs = slice(hg * DG, (hg + 1) * DG)
        in_maps.append({
            "xT": _to_bf16(x[b].T),
            "xn": _to_bf16(x[b]),
            "wq": _to_bf16(Wq[:, cs] * SCALE),
            "wk": _to_bf16(Wk[:, cs]),
            "wv": _to_bf16(Wv[:, cs]),
            "wo": _to_bf16(Wo[cs, :]),
            "pkv": pkv_b,
        })
    return in_maps


def _to_bf16(a):
    import ml_dtypes
    return np.ascontiguousarray(np.asarray(a, np.float32)).astype(ml_dtypes.bfloat16)


def gather_output(results):
    outs = [np.asarray(results[c]["yT"], np.float32) for c in range(NCORES)]
    y = np.stack([(outs[HG * b] + outs[HG * b + 1]).T for b in range(B)])
    return np.ascontiguousarray(y, np.float32)


def kernel(x, Wq, Wk, Wv, Wo, proj_k, proj_v, _trace=False, _trace_kwargs=None):
    nc = _get_nc()
    in_maps = make_in_maps(x, Wq, Wk, Wv, Wo, proj_k, proj_v)
    res = run_bass_kernel_spmd(
        nc, in_maps, core_ids=list(range(NCORES)),
        trace=_trace, **(_trace_kwargs or {}),
    )
    out = gather_output(res.results)
    if _trace:
        return out, res
    return out
